# revision 8
# baseline (speedup 1.0000x reference)
"""Trainium2 Bass kernel for nn_AVGAE (3-layer GAT variational graph
autoencoder, N=4096) on 8 NeuronCores.

Sharding: 1D row partition of the N x N attention/score matrices — core k
owns output rows [512k, 512k+512). Small per-node features are all-gathered
between layers (AllGather over internal DRAM tiles).

Key algebraic restructuring (no elementwise transcendentals over N x N):
  exp(leaky_relu(f1_i + f2_j, a)) = max(A_i*B_j, C_i*D_j)
  with A=exp(f1), B=exp(f2), C=exp(a*f1), D=exp(a*f2)
so each N x N score tile is built with vector ALU ops only (outer-product
scalar muls + max + mask mul), all bf16, then consumed directly by the
tensor engine as attention weights.  Softmax denominators come for free as
a ones-column in the attention rhs (exp(MASK_VAL) == 0 exactly in fp32, so
masked entries contribute 0 to numerator and denominator, matching the
reference softmax).

All per-node "h" quantities of layers 1/2 are linear images of layer-0
attention output, so the layer-0 attention rhs carries
[h0@W1 | h0@W2 | per-layer score vectors | ones] and hidden itself is never
materialized.  Host precomputes the folded weight matrix; the device-side
first matmul is X_own @ Wbig.

Layers 1 and 2 run interleaved and produce TRANSPOSED outputs
(lhsT = small rhs columns, moving operand = the P tile, N=512): one matmul
per (j-tile, layer) instead of four, one PSUM bank each, and Z is produced
directly in [H2, node] layout for the Z Z^T decoder (no transposes).
"""

import numpy as np
import ml_dtypes

import concourse.bass as bass
import concourse.mybir as mybir
import concourse.tile as tile
from concourse import bacc
from concourse.bass import ts
from concourse.bass_utils import run_bass_kernel_spmd
from concourse.masks import make_identity

F32 = mybir.dt.float32
F32R = mybir.dt.float32r
BF16 = mybir.dt.bfloat16
AF = mybir.ActivationFunctionType
OP = mybir.AluOpType

N = 4096
INPUT_DIM = 512
H1 = 256
H2 = 64
ALPHA = 0.2
NCORES = 8
NB = N // NCORES          # 512 rows per core
IT = NB // 128            # 4 i-tiles per core
JT = N // 128             # 32 j-tiles

# G (layer-0 gathered rhs) column layout, width 136:
#   0:64 u1 | 64:128 u2 | 128 p1a | 129 p1b | 130 p2a | 131 p2b
#   | 132 ones | 133 B0 | 134 D0 | 135 pad
GW = 136
# G1 (layers 1+2 gathered rhs) column layout, width 136:
#   0:64 h1 | 64 ones | 65 B1 | 66 D1 | 67:131 h2 | 131 ones
#   | 132 B2 | 133 D2 | 134:136 pad
G1W = 136

# which j-tiles route their mask-multiply to GpSimd (probe; tune from trace)
GPS_MASK_EVERY = 4


def build_program():
    nc = bacc.Bacc("TRN2", target_bir_lowering=False, debug=False,
                   num_devices=NCORES)

    xt = nc.dram_tensor("xt", [INPUT_DIM, NB], F32R, kind="ExternalInput").ap()
    wbig = nc.dram_tensor("wbig", [INPUT_DIM, 134], F32R,
                          kind="ExternalInput").ap()
    maskT = nc.dram_tensor("maskT", [N, NB], BF16, kind="ExternalInput").ap()
    noiseT = nc.dram_tensor("noiseT", [H2, NB], F32, kind="ExternalInput").ap()
    apred = nc.dram_tensor("apred", [NB, N], F32, kind="ExternalOutput").ap()

    rg = [list(range(NCORES))]

    with tile.TileContext(nc) as tc, \
         tc.tile_pool(name="perm", bufs=1) as perm, \
         tc.tile_pool(name="gdram", bufs=1, space="DRAM") as gdram:

        # ---------- long-lived tiles ----------
        ident = perm.tile([128, 128], F32)
        make_identity(nc, ident)
        ones1 = perm.tile([1, 128], BF16)
        nc.vector.memset(ones1, 1.0)
        onesr = perm.tile([1, 64], F32R)
        ones64f = perm.tile([1, 64], F32)
        nc.vector.memset(ones64f, 1.0)
        nc.scalar.activation(onesr, ones64f, AF.Copy)

        mask_t = [perm.tile([128, NB], BF16, tag=f"mask{t}", name=f"mask{t}")
                  for t in range(JT)]
        for t in range(JT):
            nc.sync.dma_start(out=mask_t[t], in_=maskT[ts(t, 128), :])

        bc0a = perm.tile([128, NB], BF16)
        bc0c = perm.tile([128, NB], BF16)
        bc1a = perm.tile([128, NB], BF16)
        bc1c = perm.tile([128, NB], BF16)
        bc2a = perm.tile([128, NB], BF16)
        bc2c = perm.tile([128, NB], BF16)
        bd0 = perm.tile([128, JT, 2], F32)           # f32 B0/D0 scalar cols
        bd12 = perm.tile([128, 2, JT, 2], F32)       # f32 B/D cols layers 1,2
        r0_sb = perm.tile([128, JT, GW], BF16)       # gathered layer-0 rhs
        r1_sb = perm.tile([128, JT, G1W], BF16)      # gathered layer-1/2 rhs
        noiseT_sb = perm.tile([64, NB], F32)
        nc.sync.dma_start(out=noiseT_sb, in_=noiseT)
        zt_own = perm.tile([64, NB], F32R)
        ztb = perm.tile([64, NCORES, NB], F32R)

        g_in = gdram.tile([NB, GW], BF16)
        g_out = gdram.tile([N, GW], BF16, addr_space="Shared")
        g1_in = gdram.tile([NB, G1W], BF16)
        g1_out = gdram.tile([N, G1W], BF16, addr_space="Shared")
        ztg_in = gdram.tile([64, NB], F32R)
        ztg_out = gdram.tile([NCORES * 64, NB], F32R, addr_space="Shared")

        # ---------------- stage A: builder  R0_own = X_own @ Wbig ---------
        with tc.tile_pool(name="bld_sb", bufs=2) as bsb, \
             tc.tile_pool(name="bld_ps", bufs=2, space="PSUM") as bps:

            xt_sb = bsb.tile([128, 4, NB], F32R, tag="xt_sb")
            nc.sync.dma_start(out=xt_sb,
                              in_=xt.rearrange("(k p) n -> p k n", p=128))
            wb_sb = bsb.tile([128, 4, 134], F32R, tag="wb_sb")
            nc.sync.dma_start(out=wb_sb,
                              in_=wbig.rearrange("(k p) c -> p k c", p=128))

            a0row = bsb.tile([1, NB], BF16, tag="a0row")
            c0row = bsb.tile([1, NB], BF16, tag="c0row")

            for s in range(IT):
                psA = bps.tile([128, 134], F32, tag="psA")
                for k in range(4):
                    nc.tensor.matmul(psA, lhsT=xt_sb[:, k, ts(s, 128)],
                                     rhs=wb_sb[:, k, :],
                                     start=(k == 0), stop=(k == 3))
                gown = bsb.tile([128, GW], BF16, tag="gown")
                nc.scalar.activation(gown[:, 0:132], psA[:, 0:132], AF.Copy)
                nc.vector.memset(gown[:, 132:133], 1.0)
                nc.scalar.activation(gown[:, 133:134], psA[:, 133:134], AF.Exp)
                nc.scalar.activation(gown[:, 134:135], psA[:, 133:134], AF.Exp,
                                     scale=ALPHA)
                nc.vector.memset(gown[:, 135:136], 0.0)
                nc.sync.dma_start(out=g_in[ts(s, 128), :], in_=gown)

                # f10 column -> exp'd rows (A0 / C0)
                fcol = bsb.tile([128, 1], F32, tag="fcol")
                nc.scalar.activation(fcol, psA[:, 132:133], AF.Copy)
                psT = bps.tile([1, 128], F32, tag="psT")
                nc.tensor.transpose(psT, fcol, ident)
                nc.scalar.activation(a0row[0:1, ts(s, 128)], psT, AF.Exp)
                nc.scalar.activation(c0row[0:1, ts(s, 128)], psT, AF.Exp,
                                     scale=ALPHA)

            nc.gpsimd.collective_compute(
                "AllGather", OP.bypass, replica_groups=rg,
                ins=[g_in.opt()], outs=[g_out.opt()])

            for dst, row in ((bc0a, a0row), (bc0c, c0row)):
                psB = bps.tile([128, NB], F32, tag="psB")
                nc.tensor.matmul(psB, lhsT=ones1, rhs=row, start=True,
                                 stop=True)
                nc.scalar.activation(dst, psB, AF.Copy)

            nc.sync.dma_start(out=r0_sb,
                              in_=g_out.rearrange("(t p) c -> p t c", p=128))
            nc.vector.tensor_copy(bd0, r0_sb[:, :, 133:135])

        # ---------------- stage C: layer-0 attention pass ------------------
        with tc.tile_pool(name="p0_ps", bufs=1, space="PSUM") as p0ps, \
             tc.tile_pool(name="p0_v", bufs=4) as vp:

            ps0 = [p0ps.tile([128, 133], F32, tag=f"ps0_{s}",
                             name=f"ps0_{s}") for s in range(IT)]
            for t in range(JT):
                t1 = vp.tile([128, NB], BF16, tag="t1")
                nc.scalar.activation(t1, bc0a, AF.Copy, scale=bd0[:, t, 0:1])
                t2 = vp.tile([128, NB], BF16, tag="t2")
                if t % 2 == 0:
                    nc.gpsimd.tensor_scalar_mul(t2, bc0c, bd0[:, t, 1:2])
                else:
                    nc.vector.tensor_scalar_mul(t2, bc0c, bd0[:, t, 1:2])
                t3 = vp.tile([128, NB], BF16, tag="t3")
                nc.vector.tensor_tensor(t3, t1, t2, op=OP.max)
                pt = vp.tile([128, NB], BF16, tag="pt")
                nc.vector.tensor_tensor(pt, t3, mask_t[t], op=OP.mult)
                for s in range(IT):
                    nc.tensor.matmul(ps0[s], lhsT=pt[:, ts(s, 128)],
                                     rhs=r0_sb[:, t, 0:133],
                                     start=(t == 0), stop=(t == JT - 1))

            # ---------------- stage D: normalize + build G1 ----------------
            with tc.tile_pool(name="d_sb", bufs=2) as dsb, \
                 tc.tile_pool(name="d_ps", bufs=1, space="PSUM") as dps:

                rows12 = dsb.tile([1, 4, NB], BF16, tag="rows12", bufs=1)

                for s in range(IT):
                    r0c = dsb.tile([128, 1], F32, tag="r0c")
                    nc.vector.reciprocal(r0c, ps0[s][:, 132:133])
                    r0a = dsb.tile([128, 1], F32, tag="r0a")
                    nc.vector.tensor_scalar_mul(r0a, r0c, ALPHA)

                    g1own = dsb.tile([128, G1W], BF16, tag="g1own")
                    nc.scalar.activation(g1own[:, 0:64], ps0[s][:, 0:64],
                                         AF.Copy, scale=r0c)
                    nc.vector.memset(g1own[:, 64:65], 1.0)
                    nc.scalar.activation(g1own[:, 65:66], ps0[s][:, 129:130],
                                         AF.Exp, scale=r0c)
                    nc.scalar.activation(g1own[:, 66:67], ps0[s][:, 129:130],
                                         AF.Exp, scale=r0a)
                    nc.scalar.activation(g1own[:, 67:131], ps0[s][:, 64:128],
                                         AF.Copy, scale=r0c)
                    nc.vector.memset(g1own[:, 131:132], 1.0)
                    nc.scalar.activation(g1own[:, 132:133], ps0[s][:, 131:132],
                                         AF.Exp, scale=r0c)
                    nc.scalar.activation(g1own[:, 133:134], ps0[s][:, 131:132],
                                         AF.Exp, scale=r0a)
                    nc.vector.memset(g1own[:, 134:136], 0.0)
                    nc.sync.dma_start(out=g1_in[ts(s, 128), :], in_=g1own)

                    # f1' (col 128) and f1'' (col 130) -> exp'd rows
                    for li, col in ((0, 128), (2, 130)):
                        fcl = dsb.tile([128, 1], F32, tag="fcl")
                        nc.scalar.activation(fcl, ps0[s][:, col:col + 1],
                                             AF.Copy, scale=r0c)
                        psT2 = dps.tile([1, 128], F32, tag="psT2")
                        nc.tensor.transpose(psT2, fcl, ident)
                        nc.scalar.activation(rows12[0:1, li, ts(s, 128)],
                                             psT2, AF.Exp)
                        nc.scalar.activation(rows12[0:1, li + 1, ts(s, 128)],
                                             psT2, AF.Exp, scale=ALPHA)

                nc.gpsimd.collective_compute(
                    "AllGather", OP.bypass, replica_groups=rg,
                    ins=[g1_in.opt()], outs=[g1_out.opt()])

                for i, dst in enumerate((bc1a, bc1c, bc2a, bc2c)):
                    psB2 = dps.tile([128, NB], F32, tag="psB2")
                    nc.tensor.matmul(psB2, lhsT=ones1,
                                     rhs=rows12[0:1, i, :], start=True,
                                     stop=True)
                    nc.scalar.activation(dst, psB2, AF.Copy)

                nc.sync.dma_start(
                    out=r1_sb, in_=g1_out.rearrange("(t p) c -> p t c", p=128))
                nc.vector.tensor_copy(bd12[:, 0, :, :], r1_sb[:, :, 65:67])
                nc.vector.tensor_copy(bd12[:, 1, :, :], r1_sb[:, :, 132:134])

        # -------- stage E: layers 1+2, interleaved, transposed outputs -----
        # psT[c, i] = sum_j G1[j, c] * P[j, i]; row 64 = denominator.
        with tc.tile_pool(name="e_ps", bufs=1, space="PSUM") as eps, \
             tc.tile_pool(name="e_v", bufs=4) as vpl, \
             tc.tile_pool(name="e_sb", bufs=1) as esb:

            ps1T = eps.tile([65, NB], F32, tag="ps1T")
            ps2T = eps.tile([65, NB], F32, tag="ps2T")
            for t in range(JT):
                for layer, psT_, bca, bcc, c0 in (
                        (1, ps1T, bc1a, bc1c, 0),
                        (2, ps2T, bc2a, bc2c, 67)):
                    t1 = vpl.tile([128, NB], BF16, tag=f"t1_{layer}")
                    nc.vector.tensor_scalar_mul(t1, bca,
                                                bd12[:, layer - 1, t, 0:1])
                    t2 = vpl.tile([128, NB], BF16, tag=f"t2_{layer}")
                    nc.vector.tensor_scalar_mul(t2, bcc,
                                                bd12[:, layer - 1, t, 1:2])
                    t3 = vpl.tile([128, NB], BF16, tag=f"t3_{layer}")
                    nc.vector.tensor_tensor(t3, t1, t2, op=OP.max)
                    pt = vpl.tile([128, NB], BF16, tag=f"pt_{layer}")
                    if t % GPS_MASK_EVERY == 0:
                        nc.gpsimd.tensor_tensor(pt, t3, mask_t[t], op=OP.mult)
                    else:
                        nc.vector.tensor_tensor(pt, t3, mask_t[t], op=OP.mult)
                    nc.tensor.matmul(psT_, lhsT=r1_sb[:, t, c0:c0 + 65],
                                     rhs=pt,
                                     start=(t == 0), stop=(t == JT - 1))

            # ---- Z^T = noise^T * exp(logstd^T) + mean^T, all row-scaled ----
            r1row = esb.tile([1, NB], F32)
            nc.vector.reciprocal(r1row, ps1T[64:65, :])
            r2row = esb.tile([1, NB], F32)
            nc.vector.reciprocal(r2row, ps2T[64:65, :])
            r1r = esb.tile([1, NB], F32R)
            nc.scalar.activation(r1r, r1row, AF.Copy)
            r2r = esb.tile([1, NB], F32R)
            nc.scalar.activation(r2r, r2row, AF.Copy)

            r1bc = esb.tile([64, NB], F32)
            r2bc = esb.tile([64, NB], F32)
            for rr, rbc in ((r1r, r1bc), (r2r, r2bc)):
                psBC = eps.tile([64, NB], F32, tag="psBC", bufs=2)
                nc.tensor.matmul(psBC, lhsT=onesr, rhs=rr, start=True,
                                 stop=True)
                nc.scalar.activation(rbc, psBC, AF.Copy)

            meanT = esb.tile([64, NB], F32)
            nc.vector.tensor_tensor(meanT, ps1T[0:64, :], r1bc, op=OP.mult)
            ltT = esb.tile([64, NB], F32)
            nc.vector.tensor_tensor(ltT, ps2T[0:64, :], r2bc, op=OP.mult)
            eT = esb.tile([64, NB], F32)
            nc.scalar.activation(eT, ltT, AF.Exp)
            zmT = esb.tile([64, NB], F32)
            nc.vector.tensor_tensor(zmT, eT, noiseT_sb, op=OP.mult)
            zT = esb.tile([64, NB], F32)
            nc.vector.tensor_tensor(zT, zmT, meanT, op=OP.add)
            nc.scalar.activation(zt_own, zT, AF.Copy)

        # ---------------- stage F: gather Z^T -----------------------------
        nc.sync.dma_start(out=ztg_in, in_=zt_own)
        nc.gpsimd.collective_compute(
            "AllGather", OP.bypass, replica_groups=rg,
            ins=[ztg_in.opt()], outs=[ztg_out.opt()])
        nc.sync.dma_start(
            out=ztb, in_=ztg_out.rearrange("(b p) i -> p b i", p=64))

        # ---------------- stage G: decoder sigmoid(Z @ Z^T) ----------------
        with tc.tile_pool(name="dec_ps", bufs=4, space="PSUM") as decps, \
             tc.tile_pool(name="dec_sb", bufs=4) as decsb:
            for s in range(IT):
                for b in range(NCORES):
                    psD = decps.tile([128, NB], F32, tag="psD")
                    nc.tensor.matmul(psD, lhsT=zt_own[:, ts(s, 128)],
                                     rhs=ztb[:, b, :], start=True,
                                     stop=True)
                    osb = decsb.tile([128, NB], F32, tag="osb")
                    nc.scalar.activation(osb, psD, AF.Sigmoid)
                    nc.sync.dma_start(
                        out=apred[ts(s, 128), ts(b, NB)], in_=osb)

    nc.compile()
    return nc


_program = None


def _get_program():
    global _program
    if _program is None:
        _program = build_program()
    return _program


def kernel(X, adj, noise, W0, a0, W1, a1, W2, a2, _trace=False):
    X = np.asarray(X, dtype=np.float32)
    adj = np.asarray(adj)
    noise = np.asarray(noise, dtype=np.float32)
    W0 = np.asarray(W0, dtype=np.float32)
    a0 = np.asarray(a0, dtype=np.float32)
    W1 = np.asarray(W1, dtype=np.float32)
    a1 = np.asarray(a1, dtype=np.float32)
    W2 = np.asarray(W2, dtype=np.float32)
    a2 = np.asarray(a2, dtype=np.float32)

    # folded weight matrix [512, 134]
    u1 = W0 @ W1
    u2 = W0 @ W2
    wbig = np.concatenate([
        u1, u2,
        u1 @ a1[:H2], u1 @ a1[H2:],
        u2 @ a2[:H2], u2 @ a2[H2:],
        W0 @ a0[:H1], W0 @ a0[H1:],
    ], axis=1).astype(np.float32)

    maskT = adj.astype(ml_dtypes.bfloat16).T  # 0/1, exact in bf16

    in_maps = []
    for k in range(NCORES):
        sl = slice(k * NB, (k + 1) * NB)
        in_maps.append({
            "xt": np.ascontiguousarray(X[sl].T),
            "wbig": wbig,
            "maskT": np.ascontiguousarray(maskT[:, sl]),
            "noiseT": np.ascontiguousarray(noise[sl].T),
        })

    nc = _get_program()
    res = run_bass_kernel_spmd(nc, in_maps, core_ids=list(range(NCORES)),
                               trace=_trace)
    out = np.concatenate([res.results[k]["apred"] for k in range(NCORES)],
                         axis=0)
    if _trace:
        kernel.last_results = res
    return out


# revision 10
# speedup vs baseline: 1.1616x; 1.1616x over previous
"""Trainium2 Bass kernel for nn_AVGAE (3-layer GAT variational graph
autoencoder, N=4096) on 8 NeuronCores.

Sharding: 1D row partition of the N x N attention/score matrices — core k
owns output rows [512k, 512k+512). Small per-node features are all-gathered
between layers (AllGather over internal DRAM tiles).

Key algebraic restructuring (no elementwise transcendentals over N x N):
  exp(leaky_relu(f1_i + f2_j, a)) = max(A_i*B_j, C_i*D_j)
  with A=exp(f1), B=exp(f2), C=exp(a*f1), D=exp(a*f2)
so each N x N score tile is built with vector ALU ops only (outer-product
scalar muls + max + mask mul), all bf16, then consumed directly by the
tensor engine as attention weights.  Softmax denominators come for free as
a ones-column in the attention rhs (exp(MASK_VAL) == 0 exactly in fp32, so
masked entries contribute 0 to numerator and denominator, matching the
reference softmax).

All per-node "h" quantities of layers 1/2 are linear images of layer-0
attention output, so the layer-0 attention rhs carries
[h0@W1 | h0@W2 | per-layer score vectors | ones] and hidden itself is never
materialized.  Host precomputes the folded weight matrix; the device-side
first matmul is X_own @ Wbig.

Layers 1 and 2 run interleaved and produce TRANSPOSED outputs
(lhsT = small rhs columns, moving operand = the P tile, N=512): one matmul
per (j-tile, layer) instead of four, one PSUM bank each, and Z is produced
directly in [H2, node] layout for the Z Z^T decoder (no transposes).
"""

import numpy as np
import ml_dtypes

import concourse.bass as bass
import concourse.mybir as mybir
import concourse.tile as tile
from concourse import bacc
from concourse.bass import ts
from concourse.bass_utils import run_bass_kernel_spmd
from concourse.masks import make_identity

F32 = mybir.dt.float32
F32R = mybir.dt.float32r
BF16 = mybir.dt.bfloat16
AF = mybir.ActivationFunctionType
OP = mybir.AluOpType

N = 4096
INPUT_DIM = 512
H1 = 256
H2 = 64
ALPHA = 0.2
NCORES = 8
NB = N // NCORES          # 512 rows per core
IT = NB // 128            # 4 i-tiles per core
JT = N // 128             # 32 j-tiles

# G (layer-0 gathered rhs) column layout, width 136:
#   0:64 u1 | 64:128 u2 | 128 p1a | 129 p1b | 130 p2a | 131 p2b
#   | 132 ones | 133 B0 | 134 D0 | 135 pad
GW = 136
# G1 (layers 1+2 gathered rhs) column layout, width 136:
#   0:64 h1 | 64 ones | 65 B1 | 66 D1 | 67:131 h2 | 131 ones
#   | 132 B2 | 133 D2 | 134:136 pad
G1W = 136

# which j-tiles route their mask-multiply to GpSimd (probe; tune from trace)
GPS_MASK_EVERY = 4


def build_program():
    nc = bacc.Bacc("TRN2", target_bir_lowering=False, debug=False,
                   num_devices=NCORES)

    xt = nc.dram_tensor("xt", [INPUT_DIM, NB], F32R, kind="ExternalInput").ap()
    wbig = nc.dram_tensor("wbig", [INPUT_DIM, 134], F32R,
                          kind="ExternalInput").ap()
    maskT = nc.dram_tensor("maskT", [N, NB], BF16, kind="ExternalInput").ap()
    noiseT = nc.dram_tensor("noiseT", [H2, NB], F32, kind="ExternalInput").ap()
    apred = nc.dram_tensor("apred", [NB, N], F32, kind="ExternalOutput").ap()
    # scratch for diagonal blocks computed during the Z^T gather (host ignores;
    # the post-gather loop recomputes them into apred)
    apred_d = nc.dram_tensor("apred_d", [NB, NB], F32,
                             kind="ExternalOutput").ap()

    rg = [list(range(NCORES))]

    with tile.TileContext(nc) as tc, \
         tc.tile_pool(name="perm", bufs=1) as perm, \
         tc.tile_pool(name="gdram", bufs=1, space="DRAM") as gdram:

        # ---------- long-lived tiles ----------
        ident = perm.tile([128, 128], F32)
        make_identity(nc, ident)
        ones1 = perm.tile([1, 128], BF16)
        nc.vector.memset(ones1, 1.0)
        onesr = perm.tile([1, 64], F32R)
        ones64f = perm.tile([1, 64], F32)
        nc.vector.memset(ones64f, 1.0)
        nc.scalar.activation(onesr, ones64f, AF.Copy)

        mask_t = [perm.tile([128, NB], BF16, tag=f"mask{t}", name=f"mask{t}")
                  for t in range(JT)]

        bc0a = perm.tile([128, NB], BF16)
        bc0c = perm.tile([128, NB], BF16)
        bc1a = perm.tile([128, NB], BF16)
        bc1c = perm.tile([128, NB], BF16)
        bc2a = perm.tile([128, NB], BF16)
        bc2c = perm.tile([128, NB], BF16)
        bd0 = perm.tile([128, JT, 2], F32)           # f32 B0/D0 scalar cols
        bd12 = perm.tile([128, 2, JT, 2], F32)       # f32 B/D cols layers 1,2
        r0_sb = perm.tile([128, JT, GW], BF16)       # gathered layer-0 rhs
        r1_sb = perm.tile([128, JT, G1W], BF16)      # gathered layer-1/2 rhs
        noiseT_sb = perm.tile([64, NB], F32)
        nc.sync.dma_start(out=noiseT_sb, in_=noiseT)
        zt_own = perm.tile([64, NB], F32R)
        ztb = perm.tile([64, NCORES, NB], F32R)

        sync_in = gdram.tile([1, 16], F32)
        sync_out = gdram.tile([NCORES, 16], F32, addr_space="Shared")
        g_in = gdram.tile([NB, GW], BF16)
        g_out = gdram.tile([N, GW], BF16, addr_space="Shared")
        g1_in = gdram.tile([NB, G1W], BF16)
        g1_out = gdram.tile([N, G1W], BF16, addr_space="Shared")
        ztg_in = gdram.tile([64, NB], F32R)
        ztg_out = gdram.tile([NCORES * 64, NB], F32R, addr_space="Shared")

        # ---------------- stage A: builder  R0_own = X_own @ Wbig ---------
        with tc.tile_pool(name="bld_sb", bufs=2) as bsb, \
             tc.tile_pool(name="bld_ps", bufs=2, space="PSUM") as bps:

            xt_sb = bsb.tile([128, 4, NB], F32R, tag="xt_sb")
            nc.sync.dma_start(out=xt_sb,
                              in_=xt.rearrange("(k p) n -> p k n", p=128))
            wb_sb = bsb.tile([128, 4, 134], F32R, tag="wb_sb")
            nc.sync.dma_start(out=wb_sb,
                              in_=wbig.rearrange("(k p) c -> p k c", p=128))

            a0row = bsb.tile([1, NB], BF16, tag="a0row")
            c0row = bsb.tile([1, NB], BF16, tag="c0row")

            # early barrier: absorb inter-core launch skew off the critical
            # path so the real gathers only pay the collective floor
            zt16 = bsb.tile([1, 16], F32, tag="zt16")
            nc.vector.memset(zt16, 0.0)
            nc.sync.dma_start(out=sync_in, in_=zt16)
            nc.gpsimd.collective_compute(
                "AllGather", OP.bypass, replica_groups=rg,
                ins=[sync_in.opt()], outs=[sync_out.opt()])

            for t in range(JT):
                nc.sync.dma_start(out=mask_t[t], in_=maskT[ts(t, 128), :])

            for s in range(IT):
                psA = bps.tile([128, 134], F32, tag="psA")
                for k in range(4):
                    nc.tensor.matmul(psA, lhsT=xt_sb[:, k, ts(s, 128)],
                                     rhs=wb_sb[:, k, :],
                                     start=(k == 0), stop=(k == 3))
                gown = bsb.tile([128, GW], BF16, tag="gown")
                nc.scalar.activation(gown[:, 0:132], psA[:, 0:132], AF.Copy)
                nc.vector.memset(gown[:, 132:133], 1.0)
                nc.scalar.activation(gown[:, 133:134], psA[:, 133:134], AF.Exp)
                nc.scalar.activation(gown[:, 134:135], psA[:, 133:134], AF.Exp,
                                     scale=ALPHA)
                nc.vector.memset(gown[:, 135:136], 0.0)
                nc.sync.dma_start(out=g_in[ts(s, 128), :], in_=gown)

                # f10 column -> exp'd rows (A0 / C0)
                fcol = bsb.tile([128, 1], F32, tag="fcol")
                nc.scalar.activation(fcol, psA[:, 132:133], AF.Copy)
                psT = bps.tile([1, 128], F32, tag="psT")
                nc.tensor.transpose(psT, fcol, ident)
                nc.scalar.activation(a0row[0:1, ts(s, 128)], psT, AF.Exp)
                nc.scalar.activation(c0row[0:1, ts(s, 128)], psT, AF.Exp,
                                     scale=ALPHA)

            nc.gpsimd.collective_compute(
                "AllGather", OP.bypass, replica_groups=rg,
                ins=[g_in.opt()], outs=[g_out.opt()])

            for dst, row in ((bc0a, a0row), (bc0c, c0row)):
                psB = bps.tile([128, NB], F32, tag="psB")
                nc.tensor.matmul(psB, lhsT=ones1, rhs=row, start=True,
                                 stop=True)
                nc.scalar.activation(dst, psB, AF.Copy)

            nc.sync.dma_start(out=r0_sb,
                              in_=g_out.rearrange("(t p) c -> p t c", p=128))
            nc.vector.tensor_copy(bd0, r0_sb[:, :, 133:135])

        # ---------------- stage C: layer-0 attention pass ------------------
        with tc.tile_pool(name="p0_ps", bufs=1, space="PSUM") as p0ps, \
             tc.tile_pool(name="p0_v", bufs=4) as vp:

            ps0 = [p0ps.tile([128, 133], F32, tag=f"ps0_{s}",
                             name=f"ps0_{s}") for s in range(IT)]
            for t in range(JT):
                t1 = vp.tile([128, NB], BF16, tag="t1")
                nc.scalar.activation(t1, bc0a, AF.Copy, scale=bd0[:, t, 0:1])
                t3 = vp.tile([128, NB], BF16, tag="t3")
                nc.vector.scalar_tensor_tensor(
                    t3, in0=bc0c, scalar=bd0[:, t, 1:2], in1=t1,
                    op0=OP.mult, op1=OP.max)
                pt = vp.tile([128, NB], BF16, tag="pt")
                nc.vector.tensor_tensor(pt, t3, mask_t[t], op=OP.mult)
                for s in range(IT):
                    nc.tensor.matmul(ps0[s], lhsT=pt[:, ts(s, 128)],
                                     rhs=r0_sb[:, t, 0:133],
                                     start=(t == 0), stop=(t == JT - 1))

            # ---------------- stage D: normalize + build G1 ----------------
            with tc.tile_pool(name="d_sb", bufs=2) as dsb, \
                 tc.tile_pool(name="d_ps", bufs=1, space="PSUM") as dps:

                rows12 = dsb.tile([1, 4, NB], BF16, tag="rows12", bufs=1)

                for s in range(IT):
                    r0c = dsb.tile([128, 1], F32, tag="r0c")
                    nc.vector.reciprocal(r0c, ps0[s][:, 132:133])
                    r0a = dsb.tile([128, 1], F32, tag="r0a")
                    nc.vector.tensor_scalar_mul(r0a, r0c, ALPHA)

                    g1own = dsb.tile([128, G1W], BF16, tag="g1own")
                    nc.scalar.activation(g1own[:, 0:64], ps0[s][:, 0:64],
                                         AF.Copy, scale=r0c)
                    nc.vector.memset(g1own[:, 64:65], 1.0)
                    nc.scalar.activation(g1own[:, 65:66], ps0[s][:, 129:130],
                                         AF.Exp, scale=r0c)
                    nc.scalar.activation(g1own[:, 66:67], ps0[s][:, 129:130],
                                         AF.Exp, scale=r0a)
                    nc.scalar.activation(g1own[:, 67:131], ps0[s][:, 64:128],
                                         AF.Copy, scale=r0c)
                    nc.vector.memset(g1own[:, 131:132], 1.0)
                    nc.scalar.activation(g1own[:, 132:133], ps0[s][:, 131:132],
                                         AF.Exp, scale=r0c)
                    nc.scalar.activation(g1own[:, 133:134], ps0[s][:, 131:132],
                                         AF.Exp, scale=r0a)
                    nc.vector.memset(g1own[:, 134:136], 0.0)
                    nc.sync.dma_start(out=g1_in[ts(s, 128), :], in_=g1own)

                    # f1' (col 128) and f1'' (col 130) -> exp'd rows
                    for li, col in ((0, 128), (2, 130)):
                        fcl = dsb.tile([128, 1], F32, tag="fcl")
                        nc.scalar.activation(fcl, ps0[s][:, col:col + 1],
                                             AF.Copy, scale=r0c)
                        psT2 = dps.tile([1, 128], F32, tag="psT2")
                        nc.tensor.transpose(psT2, fcl, ident)
                        nc.scalar.activation(rows12[0:1, li, ts(s, 128)],
                                             psT2, AF.Exp)
                        nc.scalar.activation(rows12[0:1, li + 1, ts(s, 128)],
                                             psT2, AF.Exp, scale=ALPHA)

                nc.gpsimd.collective_compute(
                    "AllGather", OP.bypass, replica_groups=rg,
                    ins=[g1_in.opt()], outs=[g1_out.opt()])

                for i, dst in enumerate((bc1a, bc1c, bc2a, bc2c)):
                    psB2 = dps.tile([128, NB], F32, tag="psB2")
                    nc.tensor.matmul(psB2, lhsT=ones1,
                                     rhs=rows12[0:1, i, :], start=True,
                                     stop=True)
                    nc.scalar.activation(dst, psB2, AF.Copy)

                nc.sync.dma_start(
                    out=r1_sb, in_=g1_out.rearrange("(t p) c -> p t c", p=128))
                nc.vector.tensor_copy(bd12[:, 0, :, :], r1_sb[:, :, 65:67])
                nc.vector.tensor_copy(bd12[:, 1, :, :], r1_sb[:, :, 132:134])

        # -------- stage E: layers 1+2, interleaved, transposed outputs -----
        # psT[c, i] = sum_j G1[j, c] * P[j, i]; row 64 = denominator.
        with tc.tile_pool(name="e_ps", bufs=1, space="PSUM") as eps, \
             tc.tile_pool(name="e_v", bufs=4) as vpl, \
             tc.tile_pool(name="e_sb", bufs=1) as esb:

            ps1T = eps.tile([65, NB], F32, tag="ps1T")
            ps2T = eps.tile([65, NB], F32, tag="ps2T")
            for t in range(JT):
                for layer, psT_, bca, bcc, c0 in (
                        (1, ps1T, bc1a, bc1c, 0),
                        (2, ps2T, bc2a, bc2c, 67)):
                    t1 = vpl.tile([128, NB], BF16, tag=f"t1_{layer}")
                    nc.vector.tensor_scalar_mul(t1, bca,
                                                bd12[:, layer - 1, t, 0:1])
                    t2 = vpl.tile([128, NB], BF16, tag=f"t2_{layer}")
                    nc.vector.tensor_scalar_mul(t2, bcc,
                                                bd12[:, layer - 1, t, 1:2])
                    t3 = vpl.tile([128, NB], BF16, tag=f"t3_{layer}")
                    nc.vector.tensor_tensor(t3, t1, t2, op=OP.max)
                    pt = vpl.tile([128, NB], BF16, tag=f"pt_{layer}")
                    if t % GPS_MASK_EVERY == 0:
                        nc.gpsimd.tensor_tensor(pt, t3, mask_t[t], op=OP.mult)
                    else:
                        nc.vector.tensor_tensor(pt, t3, mask_t[t], op=OP.mult)
                    nc.tensor.matmul(psT_, lhsT=r1_sb[:, t, c0:c0 + 65],
                                     rhs=pt,
                                     start=(t == 0), stop=(t == JT - 1))

            # ---- Z^T = noise^T * exp(logstd^T) + mean^T, all row-scaled ----
            r1row = esb.tile([1, NB], F32)
            nc.vector.reciprocal(r1row, ps1T[64:65, :])
            r2row = esb.tile([1, NB], F32)
            nc.vector.reciprocal(r2row, ps2T[64:65, :])
            r1r = esb.tile([1, NB], F32R)
            nc.scalar.activation(r1r, r1row, AF.Copy)
            r2r = esb.tile([1, NB], F32R)
            nc.scalar.activation(r2r, r2row, AF.Copy)

            r1bc = esb.tile([64, NB], F32)
            r2bc = esb.tile([64, NB], F32)
            for rr, rbc in ((r1r, r1bc), (r2r, r2bc)):
                psBC = eps.tile([64, NB], F32, tag="psBC", bufs=2)
                nc.tensor.matmul(psBC, lhsT=onesr, rhs=rr, start=True,
                                 stop=True)
                nc.scalar.activation(rbc, psBC, AF.Copy)

            meanT = esb.tile([64, NB], F32)
            nc.vector.tensor_tensor(meanT, ps1T[0:64, :], r1bc, op=OP.mult)
            ltT = esb.tile([64, NB], F32)
            nc.vector.tensor_tensor(ltT, ps2T[0:64, :], r2bc, op=OP.mult)
            eT = esb.tile([64, NB], F32)
            nc.scalar.activation(eT, ltT, AF.Exp)
            zmT = esb.tile([64, NB], F32)
            nc.vector.tensor_tensor(zmT, eT, noiseT_sb, op=OP.mult)
            zT = esb.tile([64, NB], F32)
            nc.vector.tensor_tensor(zT, zmT, meanT, op=OP.add)
            nc.scalar.activation(zt_own, zT, AF.Copy)

        # ---------------- stage F: gather Z^T -----------------------------
        nc.sync.dma_start(out=ztg_in, in_=zt_own)
        nc.gpsimd.collective_compute(
            "AllGather", OP.bypass, replica_groups=rg,
            ins=[ztg_in.opt()], outs=[ztg_out.opt()])
        nc.sync.dma_start(
            out=ztb, in_=ztg_out.rearrange("(b p) i -> p b i", p=64))

        # ---------------- stage G: decoder sigmoid(Z @ Z^T) ----------------
        with tc.tile_pool(name="dec_ps", bufs=4, space="PSUM") as decps, \
             tc.tile_pool(name="dec_sb", bufs=4) as decsb:
            def dec_block(s, b, rhs, out_ap):
                psD = decps.tile([128, NB], F32, tag="psD",
                                 name=f"psD_{s}_{b}")
                nc.tensor.matmul(psD, lhsT=zt_own[:, ts(s, 128)],
                                 rhs=rhs, start=True, stop=True)
                osb = decsb.tile([128, NB], F32, tag="osb",
                                 name=f"osb_{s}_{b}")
                nc.scalar.activation(osb, psD, AF.Sigmoid)
                nc.sync.dma_start(out=out_ap, in_=osb)

            # own (diagonal) blocks first — overlap with the Z^T gather
            for s in range(IT):
                dec_block(s, 'd', zt_own, apred_d[ts(s, 128), :])
            for s in range(IT):
                for b in range(NCORES):
                    dec_block(s, b, ztb[:, b, :],
                              apred[ts(s, 128), ts(b, NB)])

    nc.compile()
    return nc


_program = None


def _get_program():
    global _program
    if _program is None:
        _program = build_program()
    return _program


def kernel(X, adj, noise, W0, a0, W1, a1, W2, a2, _trace=False):
    X = np.asarray(X, dtype=np.float32)
    adj = np.asarray(adj)
    noise = np.asarray(noise, dtype=np.float32)
    W0 = np.asarray(W0, dtype=np.float32)
    a0 = np.asarray(a0, dtype=np.float32)
    W1 = np.asarray(W1, dtype=np.float32)
    a1 = np.asarray(a1, dtype=np.float32)
    W2 = np.asarray(W2, dtype=np.float32)
    a2 = np.asarray(a2, dtype=np.float32)

    # folded weight matrix [512, 134]
    u1 = W0 @ W1
    u2 = W0 @ W2
    wbig = np.concatenate([
        u1, u2,
        u1 @ a1[:H2], u1 @ a1[H2:],
        u2 @ a2[:H2], u2 @ a2[H2:],
        W0 @ a0[:H1], W0 @ a0[H1:],
    ], axis=1).astype(np.float32)

    maskT = adj.astype(ml_dtypes.bfloat16).T  # 0/1, exact in bf16

    in_maps = []
    for k in range(NCORES):
        sl = slice(k * NB, (k + 1) * NB)
        in_maps.append({
            "xt": np.ascontiguousarray(X[sl].T),
            "wbig": wbig,
            "maskT": np.ascontiguousarray(maskT[:, sl]),
            "noiseT": np.ascontiguousarray(noise[sl].T),
        })

    nc = _get_program()
    res = run_bass_kernel_spmd(nc, in_maps, core_ids=list(range(NCORES)),
                               trace=_trace)
    out = np.concatenate([res.results[k]["apred"] for k in range(NCORES)],
                         axis=0)
    if _trace:
        kernel.last_results = res
    return out
